# revision 21
# baseline (speedup 1.0000x reference)
"""Multi-head attention (B=2, S=2048, D=1024, H=16, dk=64) on 8 Trainium2
NeuronCores via Bass/Tile.

Sharding: core c handles batch b = c//4 and head-group g = c%4 (4 heads,
256 qkv columns).  Each core computes its QKV projection slices, 4 heads of
attention, and a partial output projection against its 256-row slice of Wo.
The host sums the 4 partial outputs per batch and folds in bo and bv@Wo.

v3 design notes (vs v2 baseline at ~373us):
- All operands bf16 (x, Wq/Wk/Wv/Wo, qT/kT/vt/ex/oT); PSUM accumulates
  fp32.  Errors average out over the large contractions; halves DMA and
  SBUF traffic and enables fast-weight-load on 128-col LDWEIGHTS.
- bk dropped entirely: it shifts every score of a (q,head) row by the same
  constant, which softmax cancels exactly.
- exp was the phase-B bottleneck (ACT = 1 elem/cyc/lane -> 142us > PE work,
  starving the PE and triggering HAM 4/8 down-throttle for ~60% of the
  kernel).  Now split: kc % 4 == 3 computed on DVE with a two-term
  product-form Schraudolph (exp(s) ~ bitcast(A/2*s+B) * bitcast(A/2*s+B+64),
  opposite sawtooth phases; global sqrt(2) scale cancels in softmax), the
  other 3/4 on ACT (exact).  Both engines land ~95% busy under the PE pace.
- Deeper pipeline: scores run 2 kc ahead in a 4-bank PSUM rotation, exp one
  ahead, AV trails; PE never idles so HAM stays at 8/8 (2.4 GHz).
- Normalization: denominators via the ones-column of the AV lhsT; recip +
  broadcast as in v2 but the normalized halves DMA straight into oT (bf16).
"""

import numpy as np

P = 128
B, S, D = 2, 2048, 1024
H, DK = 16, 64
COLS = 256          # qkv columns per core (4 heads)
KC = D // P         # 8 contraction chunks for the projections
TT = 512            # token block (matmul free dim)
NJ = S // TT        # 4 token blocks
NT = S // P         # 16 token tiles
NKT = S // P        # 16 key tiles
VW = 65             # per-head AV lhsT width: 64 v-dims + ones column
VP = 72             # padded per-head stride in vt (144B, 8B-aligned writes)

# two-term product Schraudolph for exp(0.125*s) on bf16 bit patterns:
#   exp(x) = 2^(x*log2e); bf16 bits b encode 2^((b-16256)/128) up to the
#   mantissa-vs-log sawtooth.  t = 0.125*s*log2e*128; use half-scale terms
#   t/2 + B and t/2 + B + 64 whose sawtooth phases are opposite; the product
#   restores t and cancels most of the sawtooth.  The leftover sqrt(2)
#   factor is uniform and cancels in softmax.
SCHR_A = 0.125 * 1.4426950408889634 * 128.0  # = 0.125*log2(e)*128
# 16256 = 127<<7 (bf16 bits of 1.0); -7.25 bits zero the mean sawtooth error
# against ACT's exact exp so both engines mix consistently in one softmax.
SCHR_B = 16248.75
# ACT's measured throughput is ~1660ns per [128,1024] exp call (1114ns dur
# plus a ~550ns inter-instruction engine bubble), well short of the PE's
# ~1000ns/kc pace — so DVE's Schraudolph takes 7 of 16 kc.
DVE_KCS = frozenset({3, 5, 7, 9, 11, 13, 15})

_CACHE = {}


def _build():
    import concourse.bass as bass
    import concourse.tile as tile
    from concourse import bacc, mybir

    f32 = mybir.dt.float32
    f32r = mybir.dt.float32r
    bf16 = mybir.dt.bfloat16
    i16 = mybir.dt.int16
    Exp = mybir.ActivationFunctionType.Exp
    MUL = mybir.AluOpType.mult
    ADD = mybir.AluOpType.add

    nc = bacc.Bacc(
        "TRN2", target_bir_lowering=False, debug=False,
        enable_asserts=False, num_devices=8,
    )
    x_d = nc.dram_tensor("x", [S, D], bf16, kind="ExternalInput").ap()
    wq_d = nc.dram_tensor("wq", [D, COLS], bf16, kind="ExternalInput").ap()
    wk_d = nc.dram_tensor("wk", [D, COLS], bf16, kind="ExternalInput").ap()
    wv_d = nc.dram_tensor("wv", [D, COLS], bf16, kind="ExternalInput").ap()
    wo_d = nc.dram_tensor("wo", [COLS, D], bf16, kind="ExternalInput").ap()
    bq_d = nc.dram_tensor("bq", [COLS], f32, kind="ExternalInput").ap()
    out_d = nc.dram_tensor("out_t", [D, S], f32, kind="ExternalOutput").ap()

    with tile.TileContext(nc) as tc:
        with (
            tc.tile_pool(name="const", bufs=1) as const,
            tc.tile_pool(name="wpool", bufs=1) as wpool,
            tc.tile_pool(name="persist", bufs=1) as persist,
            tc.tile_pool(name="xtp", bufs=4) as xtp,
            tc.tile_pool(name="exps", bufs=36) as exps,
            tc.tile_pool(name="stage", bufs=2) as stage,
            tc.tile_pool(name="outst", bufs=4) as outst,
            tc.tile_pool(name="ps_sc", bufs=1, space="PSUM") as ps_sc,
            tc.tile_pool(name="ps_acc", bufs=2, space="PSUM") as ps_acc,
            tc.tile_pool(name="ps_u", bufs=2, space="PSUM") as ps_u,
        ):
            # ---- input DMAs: x transposes on the sync queue, weights on the
            # scalar queue (both are hwdge engines — descriptor generation for
            # the big xbar transposes is the serial cost, so split queues) ----
            xTs = {}
            for j in range(NJ):
                xTs[j] = xtp.tile([P, KC, TT], bf16, tag="xT", name=f"xT{j}")
                nc.sync.dma_start_transpose(xTs[j][:], x_d[bass.ts(j, TT), :])

            wk_sb = wpool.tile([P, KC, COLS], bf16, tag="wk")
            nc.scalar.dma_start(wk_sb[:], wk_d.rearrange("(o p) f -> p o f", p=P))
            wq_sb = wpool.tile([P, KC, COLS], bf16, tag="wq")
            nc.scalar.dma_start(wq_sb[:], wq_d.rearrange("(o p) f -> p o f", p=P))
            bq_sb = const.tile([P, 2], f32, tag="bq")
            nc.scalar.dma_start(bq_sb[:], bq_d.rearrange("(o p) -> p o", p=P))
            wv_sb = wpool.tile([P, KC, COLS], bf16, tag="wv")
            nc.scalar.dma_start(wv_sb[:], wv_d.rearrange("(o p) f -> p o f", p=P))
            wo_sb = wpool.tile([P, 2, D], bf16, tag="wo")
            nc.scalar.dma_start(wo_sb[:], wo_d.rearrange("(o p) f -> p o f", p=P))

            # ones: f32r [P, VW] for the norm broadcast; bf16 row for vt
            ones32 = const.tile([P, VW], f32, tag="ones32")
            nc.vector.memset(ones32[:], 1.0)
            ones_r = const.tile([P, VW], f32r, tag="ones_r")
            nc.vector.tensor_copy(ones_r[:], ones32[:])
            ones_bf = const.tile([P, NT * 4], bf16, tag="ones_bf")
            nc.vector.memset(ones_bf[:], 1.0)

            # preload the Exp table while DMAs run
            dummy = const.tile([P, 1], f32, tag="dummy")
            nc.scalar.activation(dummy[:], ones32[:, 0:1], Exp, scale=1.0)

            # persistent activations (all bf16)
            qT = persist.tile([P, 2, S], bf16, tag="qT")    # [qcol, tok]
            kT = persist.tile([P, 2, S], bf16, tag="kT")    # [kcol, tok]
            vt = persist.tile([P, NT, 4 * VP], bf16, tag="vt")  # [tok, h*(V|1)]
            oT = persist.tile([P, 2, S], bf16, tag="oT")    # [vdim, tok]

            vt_heads = vt[:].rearrange("p t (h c) -> p t h c", c=VP)
            nc.vector.tensor_copy(
                vt_heads[:, :, :, 64],
                ones_bf[:].rearrange("p (t h) -> p t h", h=4),
            )

            # ---- phase A: [K(j), Q(j)] per block (PE stays dense per xT
            # arrival, kT still completes early), then all V.  PSUM evacs on
            # ACT (idle here); accumulators rotate over 4 banks by borrowing
            # the ps_acc pool, so ACT's evac latency never gates a chain. ----
            nacc = 0

            def acc_tile(shape):
                nonlocal nacc
                pool = (ps_u, ps_acc)[nacc % 2]
                nacc += 1
                return pool.tile(shape, f32, tag="u" if pool is ps_u else "acc",
                                 name="pa_acc")

            for j in range(NJ):
                for (wmat, dstT, bias) in (
                    (wk_sb, kT, None),      # no bk: softmax cancels it
                    (wq_sb, qT, bq_sb),
                ):
                    for ct in range(2):
                        acc = acc_tile([P, TT])
                        for kc in range(KC):
                            nc.tensor.matmul(
                                acc[:], wmat[:, kc, bass.ts(ct, P)],
                                xTs[j][:, kc, :],
                                start=(kc == 0), stop=(kc == KC - 1),
                            )
                        if bias is not None:
                            nc.scalar.add(
                                dstT[:, ct, bass.ts(j, TT)], acc[:],
                                bias[:, ct : ct + 1],
                            )
                        else:
                            nc.scalar.copy(
                                dstT[:, ct, bass.ts(j, TT)], acc[:]
                            )

            for j in range(NJ):
                for ts4 in range(TT // P):
                    acc = acc_tile([P, COLS])
                    for kc in range(KC):
                        nc.tensor.matmul(
                            acc[:], xTs[j][:, kc, bass.ts(ts4, P)],
                            wv_sb[:, kc, :],
                            start=(kc == 0), stop=(kc == KC - 1),
                        )
                    tt = 4 * j + ts4
                    nc.scalar.copy(
                        vt_heads[:, tt, :, 0:64],
                        acc[:].rearrange("p (h c) -> p h c", c=64),
                    )

            # shared scores PSUM: 4 banks, pair-rotated (2 kc in flight)
            big_sc = ps_sc.tile([P, 4, TT], f32, tag="sc")

            # ---- phase B ----
            # Software-pipelined one block-pair deep: while (j,p)'s scores and
            # exp are produced, the AV/norm/Wo for the PREVIOUS (j,p) runs off
            # its fully-materialized ex tiles.  AV never waits on exp, so the
            # PE always has ready work and transient ACT/DVE jitter only
            # touches the 2-kc score lookahead.

            def sc_pair(j, p, kc):
                base = (2 * kc) % 4
                for i in range(2):
                    lo = 64 * i
                    nc.tensor.matmul(
                        big_sc[:, base + i, :],
                        kT[lo : lo + 64, p, bass.ts(kc, P)],
                        qT[lo : lo + 64, p, bass.ts(j, TT)],
                        start=True, stop=True,
                    )

            def exp_emit(kc):
                base = (2 * kc) % 4
                ex = exps.tile([P, 2, TT], bf16, tag="ex", name="ex")
                if kc not in DVE_KCS:
                    nc.scalar.activation(
                        ex[:], big_sc[:, base : base + 2, :], Exp, scale=0.125,
                    )
                else:
                    # Schraudolph on DVE: one tensor_scalar writes the bf16
                    # bit pattern of exp(0.125*s) as int16
                    nc.vector.tensor_scalar(
                        ex[:].bitcast(i16), big_sc[:, base : base + 2, :],
                        SCHR_A, SCHR_B, MUL, ADD,
                    )
                return ex

            def norm_wo_chunks(j, p, o_ps):
                """Emission thunks for normalize+project of a finished block.
                They get interleaved one-per-kc into the NEXT block's loop so
                the DVE/PE queues never see a burst at block boundaries."""
                state = {}
                chunks = []

                def osb_c(i):
                    osb = stage.tile([P, TT], f32r, tag="osb", name="osb")
                    nc.vector.tensor_copy(osb[0:VW, :], o_ps[i][:])
                    state[i] = osb

                def nrm_c(i):
                    osb = state[i]
                    rbc = ps_u.tile([64, TT], f32, tag="u", name="rbc")
                    nc.tensor.matmul(
                        rbc[:], ones_r[64:65, 0:64], osb[64:65, :],
                        start=True, stop=True,
                    )
                    rbs = stage.tile([64, TT], f32, tag="rbs", name="rbs")
                    nc.vector.reciprocal_approx_fast(rbs[:], rbc[:])
                    onrm = stage.tile([64, TT], bf16, tag="onrm", name="onrm")
                    nc.vector.tensor_tensor(onrm[:], osb[0:64, :], rbs[:], MUL)
                    nc.sync.dma_start(
                        oT[bass.ds(64 * i, 64), p, bass.ts(j, TT)], onrm[:]
                    )

                def wo_c(oc):
                    acc = ps_u.tile([P, TT], f32, tag="u", name="wo_acc")
                    for vc in range(2):
                        nc.tensor.matmul(
                            acc[:], wo_sb[:, vc, bass.ts(oc, P)],
                            oT[:, vc, bass.ts(j, TT)],
                            start=(vc == 0), stop=(vc == 1),
                        )
                    st = outst.tile([P, TT], f32, tag="outst", name="outst")
                    nc.vector.tensor_copy(st[:], acc[:])
                    nc.sync.dma_start(out_d[bass.ts(oc, P), bass.ts(j, TT)], st[:])

                for i in range(2):
                    chunks.append(lambda i=i: osb_c(i))
                    chunks.append(lambda i=i: nrm_c(i))
                if p == 1:
                    for oc in range(D // P):
                        chunks.append(lambda oc=oc: wo_c(oc))
                return chunks

            def av_emit(pp, pex, o_ps, kc):
                for i in range(2):
                    nc.tensor.matmul(
                        o_ps[i][:],
                        vt[:, kc, bass.ds(VP * (2 * pp + i), VW)],
                        pex[kc][:, i, :],
                        start=(kc == 0), stop=(kc == NKT - 1),
                    )

            prev = None      # (j, p, [ex tiles])
            deferred = []    # norm/Wo thunks of the block before prev
            for j in range(NJ):
                for p in range(2):
                    if prev is not None:
                        o_ps = [
                            ps_acc.tile([VW, TT], f32, tag="acc",
                                        name=f"o_ps{i}")
                            for i in range(2)
                        ]
                        pj, pp, pex = prev
                    exs = []
                    for kc in range(NKT):
                        if deferred:
                            deferred.pop(0)()
                        if prev is not None:
                            av_emit(pp, pex, o_ps, kc)
                        sc_pair(j, p, kc)
                        exs.append(exp_emit(kc))
                    if prev is not None:
                        deferred = norm_wo_chunks(pj, pp, o_ps)
                    prev = (j, p, exs)

            # drain: AV/norm/Wo for the final block-pair
            pj, pp, pex = prev
            o_ps = [
                ps_acc.tile([VW, TT], f32, tag="acc", name=f"o_ps{i}")
                for i in range(2)
            ]
            for kc in range(NKT):
                if deferred:
                    deferred.pop(0)()
                av_emit(pp, pex, o_ps, kc)
            for c in deferred:
                c()
            for c in norm_wo_chunks(pj, pp, o_ps):
                c()

    nc.compile()
    return nc


def make_in_maps(x, Wq, bq, Wk, bk, Wv, Wo):
    import ml_dtypes

    bf = ml_dtypes.bfloat16
    xb = [np.ascontiguousarray(x[b].astype(bf)) for b in range(B)]
    wqb = Wq.astype(bf)
    wkb = Wk.astype(bf)
    wvb = Wv.astype(bf)
    wob = Wo.astype(bf)

    in_maps = []
    for c in range(8):
        b, g = divmod(c, 4)
        cs = slice(COLS * g, COLS * (g + 1))
        in_maps.append({
            "x": xb[b],
            "wq": np.ascontiguousarray(wqb[:, cs]),
            "wk": np.ascontiguousarray(wkb[:, cs]),
            "wv": np.ascontiguousarray(wvb[:, cs]),
            "wo": np.ascontiguousarray(wob[cs, :]),
            "bq": np.ascontiguousarray(bq[cs].astype(np.float32)),
        })
    return in_maps


def kernel(x, Wq, bq, Wk, bk, Wv, bv, Wo, bo):
    from concourse import bass_utils

    x = np.asarray(x, dtype=np.float32)
    Wq = np.asarray(Wq, dtype=np.float32)
    Wk = np.asarray(Wk, dtype=np.float32)
    Wv = np.asarray(Wv, dtype=np.float32)
    Wo = np.asarray(Wo, dtype=np.float32)
    bq = np.asarray(bq, dtype=np.float32)
    bv = np.asarray(bv, dtype=np.float32)
    bo = np.asarray(bo, dtype=np.float32)

    if "nc" not in _CACHE:
        _CACHE["nc"] = _build()
    nc = _CACHE["nc"]

    in_maps = make_in_maps(x, Wq, bq, Wk, bk, Wv, Wo)
    res = bass_utils.run_bass_kernel_spmd(nc, in_maps, core_ids=list(range(8)))

    out = np.zeros((B, S, D), dtype=np.float32)
    for c in range(8):
        out[c // 4] += res.results[c]["out_t"].T
    out += bo + bv @ Wo
    return out


# revision 23
# speedup vs baseline: 1.0412x; 1.0412x over previous
"""Multi-head attention (B=2, S=2048, D=1024, H=16, dk=64) on 8 Trainium2
NeuronCores via Bass/Tile.

Sharding: core c handles batch b = c//4 and head-group g = c%4 (4 heads,
256 qkv columns).  Each core computes its QKV projection slices, 4 heads of
attention, and a partial output projection against its 256-row slice of Wo.
The host sums the 4 partial outputs per batch and folds in bo and bv@Wo.

v3 design notes (vs v2 baseline at ~373us):
- All operands bf16 (x, Wq/Wk/Wv/Wo, qT/kT/vt/ex/oT); PSUM accumulates
  fp32.  Errors average out over the large contractions; halves DMA and
  SBUF traffic and enables fast-weight-load on 128-col LDWEIGHTS.
- bk dropped entirely: it shifts every score of a (q,head) row by the same
  constant, which softmax cancels exactly.
- exp was the phase-B bottleneck (ACT = 1 elem/cyc/lane -> 142us > PE work,
  starving the PE and triggering HAM 4/8 down-throttle for ~60% of the
  kernel).  Now split: kc % 4 == 3 computed on DVE with a two-term
  product-form Schraudolph (exp(s) ~ bitcast(A/2*s+B) * bitcast(A/2*s+B+64),
  opposite sawtooth phases; global sqrt(2) scale cancels in softmax), the
  other 3/4 on ACT (exact).  Both engines land ~95% busy under the PE pace.
- Deeper pipeline: scores run 2 kc ahead in a 4-bank PSUM rotation, exp one
  ahead, AV trails; PE never idles so HAM stays at 8/8 (2.4 GHz).
- Normalization: denominators via the ones-column of the AV lhsT; recip +
  broadcast as in v2 but the normalized halves DMA straight into oT (bf16).
"""

import numpy as np

P = 128
B, S, D = 2, 2048, 1024
H, DK = 16, 64
COLS = 256          # qkv columns per core (4 heads)
KC = D // P         # 8 contraction chunks for the projections
TT = 512            # token block (matmul free dim)
NJ = S // TT        # 4 token blocks
NT = S // P         # 16 token tiles
NKT = S // P        # 16 key tiles
VW = 65             # per-head AV lhsT width: 64 v-dims + ones column
VP = 72             # padded per-head stride in vt (144B, 8B-aligned writes)

# two-term product Schraudolph for exp(0.125*s) on bf16 bit patterns:
#   exp(x) = 2^(x*log2e); bf16 bits b encode 2^((b-16256)/128) up to the
#   mantissa-vs-log sawtooth.  t = 0.125*s*log2e*128; use half-scale terms
#   t/2 + B and t/2 + B + 64 whose sawtooth phases are opposite; the product
#   restores t and cancels most of the sawtooth.  The leftover sqrt(2)
#   factor is uniform and cancels in softmax.
SCHR_A = 0.125 * 1.4426950408889634 * 128.0  # = 0.125*log2(e)*128
# 16256 = 127<<7 (bf16 bits of 1.0); -7.25 bits zero the mean sawtooth error
# against ACT's exact exp so both engines mix consistently in one softmax.
SCHR_B = 16248.75
# ACT's measured throughput is ~1660ns per [128,1024] exp call (1114ns dur
# plus a ~550ns inter-instruction engine bubble), well short of the PE's
# ~1000ns/kc pace — so DVE's Schraudolph takes 7 of 16 kc.
DVE_KCS = frozenset({3, 5, 7, 9, 11, 13, 15})

_CACHE = {}


def _build():
    import concourse.bass as bass
    import concourse.tile as tile
    from concourse import bacc, mybir

    f32 = mybir.dt.float32
    f32r = mybir.dt.float32r
    bf16 = mybir.dt.bfloat16
    i16 = mybir.dt.int16
    Exp = mybir.ActivationFunctionType.Exp
    MUL = mybir.AluOpType.mult
    ADD = mybir.AluOpType.add

    nc = bacc.Bacc(
        "TRN2", target_bir_lowering=False, debug=False,
        enable_asserts=False, num_devices=8,
    )
    x_d = nc.dram_tensor("x", [S, D], bf16, kind="ExternalInput").ap()
    wq_d = nc.dram_tensor("wq", [D, COLS], bf16, kind="ExternalInput").ap()
    wk_d = nc.dram_tensor("wk", [D, COLS], bf16, kind="ExternalInput").ap()
    wv_d = nc.dram_tensor("wv", [D, COLS], bf16, kind="ExternalInput").ap()
    wo_d = nc.dram_tensor("wo", [COLS, D], bf16, kind="ExternalInput").ap()
    bq_d = nc.dram_tensor("bq", [COLS], f32, kind="ExternalInput").ap()
    out_d = nc.dram_tensor("out_t", [D, S], f32, kind="ExternalOutput").ap()

    with tile.TileContext(nc) as tc:
        with (
            tc.tile_pool(name="const", bufs=1) as const,
            tc.tile_pool(name="wpool", bufs=1) as wpool,
            tc.tile_pool(name="persist", bufs=1) as persist,
            tc.tile_pool(name="xtp", bufs=4) as xtp,
            tc.tile_pool(name="exps_a", bufs=22) as exps_a,
            tc.tile_pool(name="exps_d", bufs=18) as exps_d,
            tc.tile_pool(name="stage", bufs=2) as stage,
            tc.tile_pool(name="outst", bufs=4) as outst,
            tc.tile_pool(name="ps_sc", bufs=1, space="PSUM") as ps_sc,
            tc.tile_pool(name="ps_acc", bufs=2, space="PSUM") as ps_acc,
            tc.tile_pool(name="ps_u", bufs=2, space="PSUM") as ps_u,
        ):
            # ---- input DMAs: x transposes on the sync queue, weights on the
            # scalar queue (both are hwdge engines — descriptor generation for
            # the big xbar transposes is the serial cost, so split queues) ----
            xTs = {}
            for j in range(NJ):
                xTs[j] = xtp.tile([P, KC, TT], bf16, tag="xT", name=f"xT{j}")
                nc.sync.dma_start_transpose(xTs[j][:], x_d[bass.ts(j, TT), :])

            wk_sb = wpool.tile([P, KC, COLS], bf16, tag="wk")
            nc.scalar.dma_start(wk_sb[:], wk_d.rearrange("(o p) f -> p o f", p=P))
            wq_sb = wpool.tile([P, KC, COLS], bf16, tag="wq")
            nc.scalar.dma_start(wq_sb[:], wq_d.rearrange("(o p) f -> p o f", p=P))
            bq_sb = const.tile([P, 2], f32, tag="bq")
            nc.scalar.dma_start(bq_sb[:], bq_d.rearrange("(o p) -> p o", p=P))
            wv_sb = wpool.tile([P, KC, COLS], bf16, tag="wv")
            nc.scalar.dma_start(wv_sb[:], wv_d.rearrange("(o p) f -> p o f", p=P))
            wo_sb = wpool.tile([P, 2, D], bf16, tag="wo")
            nc.scalar.dma_start(wo_sb[:], wo_d.rearrange("(o p) f -> p o f", p=P))

            # ones: f32r [P, VW] for the norm broadcast; bf16 row for vt
            ones32 = const.tile([P, VW], f32, tag="ones32")
            nc.vector.memset(ones32[:], 1.0)
            ones_r = const.tile([P, VW], f32r, tag="ones_r")
            nc.vector.tensor_copy(ones_r[:], ones32[:])
            ones_bf = const.tile([P, NT * 4], bf16, tag="ones_bf")
            nc.vector.memset(ones_bf[:], 1.0)

            # preload the Exp table while DMAs run
            dummy = const.tile([P, 1], f32, tag="dummy")
            nc.scalar.activation(dummy[:], ones32[:, 0:1], Exp, scale=1.0)

            # persistent activations (all bf16)
            qT = persist.tile([P, 2, S], bf16, tag="qT")    # [qcol, tok]
            kT = persist.tile([P, 2, S], bf16, tag="kT")    # [kcol, tok]
            vt = persist.tile([P, NT, 4 * VP], bf16, tag="vt")  # [tok, h*(V|1)]
            oT = persist.tile([P, 2, S], bf16, tag="oT")    # [vdim, tok]

            vt_heads = vt[:].rearrange("p t (h c) -> p t h c", c=VP)
            nc.vector.tensor_copy(
                vt_heads[:, :, :, 64],
                ones_bf[:].rearrange("p (t h) -> p t h", h=4),
            )

            # ---- phase A: [K(j), Q(j)] per block (PE stays dense per xT
            # arrival, kT still completes early), then all V.  PSUM evacs on
            # ACT (idle here); accumulators rotate over 4 banks by borrowing
            # the ps_acc pool, so ACT's evac latency never gates a chain. ----
            nacc = 0

            def acc_tile(shape):
                nonlocal nacc
                pool = (ps_u, ps_acc)[nacc % 2]
                nacc += 1
                return pool.tile(shape, f32, tag="u" if pool is ps_u else "acc",
                                 name="pa_acc")

            for j in range(NJ):
                for (wmat, dstT, bias) in (
                    (wk_sb, kT, None),      # no bk: softmax cancels it
                    (wq_sb, qT, bq_sb),
                ):
                    for ct in range(2):
                        acc = acc_tile([P, TT])
                        for kc in range(KC):
                            nc.tensor.matmul(
                                acc[:], wmat[:, kc, bass.ts(ct, P)],
                                xTs[j][:, kc, :],
                                start=(kc == 0), stop=(kc == KC - 1),
                            )
                        if bias is not None:
                            nc.scalar.add(
                                dstT[:, ct, bass.ts(j, TT)], acc[:],
                                bias[:, ct : ct + 1],
                            )
                        else:
                            nc.scalar.copy(
                                dstT[:, ct, bass.ts(j, TT)], acc[:]
                            )

            for j in range(NJ):
                for ts4 in range(TT // P):
                    acc = acc_tile([P, COLS])
                    for kc in range(KC):
                        nc.tensor.matmul(
                            acc[:], xTs[j][:, kc, bass.ts(ts4, P)],
                            wv_sb[:, kc, :],
                            start=(kc == 0), stop=(kc == KC - 1),
                        )
                    tt = 4 * j + ts4
                    nc.scalar.copy(
                        vt_heads[:, tt, :, 0:64],
                        acc[:].rearrange("p (h c) -> p h c", c=64),
                    )

            # shared scores PSUM: 4 banks, pair-rotated (2 kc in flight)
            big_sc = ps_sc.tile([P, 4, TT], f32, tag="sc")

            # ---- phase B ----
            # Software-pipelined one block-pair deep: while (j,p)'s scores and
            # exp are produced, the AV/norm/Wo for the PREVIOUS (j,p) runs off
            # its fully-materialized ex tiles.  AV never waits on exp, so the
            # PE always has ready work and transient ACT/DVE jitter only
            # touches the 2-kc score lookahead.

            def sc_pair(j, p, kc):
                base = (2 * kc) % 4
                for i in range(2):
                    lo = 64 * i
                    nc.tensor.matmul(
                        big_sc[:, base + i, :],
                        kT[lo : lo + 64, p, bass.ts(kc, P)],
                        qT[lo : lo + 64, p, bass.ts(j, TT)],
                        start=True, stop=True,
                    )

            def exp_emit(kc):
                # per-engine ex pools: a shared pool's ring guards would chain
                # ACT and DVE exps into one serial stream (the slot WAW crosses
                # engines); separate pools make the guards same-engine FIFO.
                base = (2 * kc) % 4
                if kc not in DVE_KCS:
                    ex = exps_a.tile([P, 2, TT], bf16, tag="exa", name="exa")
                    nc.scalar.activation(
                        ex[:], big_sc[:, base : base + 2, :], Exp, scale=0.125,
                    )
                else:
                    ex = exps_d.tile([P, 2, TT], bf16, tag="exd", name="exd")
                    # Schraudolph on DVE: one tensor_scalar writes the bf16
                    # bit pattern of exp(0.125*s) as int16
                    nc.vector.tensor_scalar(
                        ex[:].bitcast(i16), big_sc[:, base : base + 2, :],
                        SCHR_A, SCHR_B, MUL, ADD,
                    )
                return ex

            def norm_wo_chunks(j, p, o_ps):
                """Emission thunks for normalize+project of a finished block.
                They get interleaved one-per-kc into the NEXT block's loop so
                the DVE/PE queues never see a burst at block boundaries."""
                state = {}
                chunks = []

                def osb_c(i):
                    osb = stage.tile([P, TT], f32r, tag="osb", name="osb")
                    nc.vector.tensor_copy(osb[0:VW, :], o_ps[i][:])
                    state[i] = osb

                def nrm_c(i):
                    osb = state[i]
                    rbc = ps_u.tile([64, TT], f32, tag="u", name="rbc")
                    nc.tensor.matmul(
                        rbc[:], ones_r[64:65, 0:64], osb[64:65, :],
                        start=True, stop=True,
                    )
                    rbs = stage.tile([64, TT], f32, tag="rbs", name="rbs")
                    nc.vector.reciprocal_approx_fast(rbs[:], rbc[:])
                    onrm = stage.tile([64, TT], bf16, tag="onrm", name="onrm")
                    nc.vector.tensor_tensor(onrm[:], osb[0:64, :], rbs[:], MUL)
                    nc.sync.dma_start(
                        oT[bass.ds(64 * i, 64), p, bass.ts(j, TT)], onrm[:]
                    )

                def wo_c(oc):
                    acc = ps_u.tile([P, TT], f32, tag="u", name="wo_acc")
                    for vc in range(2):
                        nc.tensor.matmul(
                            acc[:], wo_sb[:, vc, bass.ts(oc, P)],
                            oT[:, vc, bass.ts(j, TT)],
                            start=(vc == 0), stop=(vc == 1),
                        )
                    st = outst.tile([P, TT], f32, tag="outst", name="outst")
                    nc.vector.tensor_copy(st[:], acc[:])
                    nc.sync.dma_start(out_d[bass.ts(oc, P), bass.ts(j, TT)], st[:])

                for i in range(2):
                    chunks.append(lambda i=i: osb_c(i))
                    chunks.append(lambda i=i: nrm_c(i))
                if p == 1:
                    for oc in range(D // P):
                        chunks.append(lambda oc=oc: wo_c(oc))
                return chunks

            def av_emit(pp, pex, o_ps, kc):
                for i in range(2):
                    nc.tensor.matmul(
                        o_ps[i][:],
                        vt[:, kc, bass.ds(VP * (2 * pp + i), VW)],
                        pex[kc][:, i, :],
                        start=(kc == 0), stop=(kc == NKT - 1),
                    )

            prev = None      # (j, p, [ex tiles])
            deferred = []    # norm/Wo thunks of the block before prev
            for j in range(NJ):
                for p in range(2):
                    if prev is not None:
                        o_ps = [
                            ps_acc.tile([VW, TT], f32, tag="acc",
                                        name=f"o_ps{i}")
                            for i in range(2)
                        ]
                        pj, pp, pex = prev
                    exs = []
                    for kc in range(NKT):
                        if deferred:
                            deferred.pop(0)()
                        if prev is not None:
                            av_emit(pp, pex, o_ps, kc)
                        sc_pair(j, p, kc)
                        exs.append(exp_emit(kc))
                    if prev is not None:
                        deferred = norm_wo_chunks(pj, pp, o_ps)
                    prev = (j, p, exs)

            # drain: AV/norm/Wo for the final block-pair
            pj, pp, pex = prev
            o_ps = [
                ps_acc.tile([VW, TT], f32, tag="acc", name=f"o_ps{i}")
                for i in range(2)
            ]
            for kc in range(NKT):
                if deferred:
                    deferred.pop(0)()
                av_emit(pp, pex, o_ps, kc)
            for c in deferred:
                c()
            for c in norm_wo_chunks(pj, pp, o_ps):
                c()

    nc.compile()
    return nc


def make_in_maps(x, Wq, bq, Wk, bk, Wv, Wo):
    import ml_dtypes

    bf = ml_dtypes.bfloat16
    xb = [np.ascontiguousarray(x[b].astype(bf)) for b in range(B)]
    wqb = Wq.astype(bf)
    wkb = Wk.astype(bf)
    wvb = Wv.astype(bf)
    wob = Wo.astype(bf)

    in_maps = []
    for c in range(8):
        b, g = divmod(c, 4)
        cs = slice(COLS * g, COLS * (g + 1))
        in_maps.append({
            "x": xb[b],
            "wq": np.ascontiguousarray(wqb[:, cs]),
            "wk": np.ascontiguousarray(wkb[:, cs]),
            "wv": np.ascontiguousarray(wvb[:, cs]),
            "wo": np.ascontiguousarray(wob[cs, :]),
            "bq": np.ascontiguousarray(bq[cs].astype(np.float32)),
        })
    return in_maps


def kernel(x, Wq, bq, Wk, bk, Wv, bv, Wo, bo):
    from concourse import bass_utils

    x = np.asarray(x, dtype=np.float32)
    Wq = np.asarray(Wq, dtype=np.float32)
    Wk = np.asarray(Wk, dtype=np.float32)
    Wv = np.asarray(Wv, dtype=np.float32)
    Wo = np.asarray(Wo, dtype=np.float32)
    bq = np.asarray(bq, dtype=np.float32)
    bv = np.asarray(bv, dtype=np.float32)
    bo = np.asarray(bo, dtype=np.float32)

    if "nc" not in _CACHE:
        _CACHE["nc"] = _build()
    nc = _CACHE["nc"]

    in_maps = make_in_maps(x, Wq, bq, Wk, bk, Wv, Wo)
    res = bass_utils.run_bass_kernel_spmd(nc, in_maps, core_ids=list(range(8)))

    out = np.zeros((B, S, D), dtype=np.float32)
    for c in range(8):
        out[c // 4] += res.results[c]["out_t"].T
    out += bo + bv @ Wo
    return out
